# revision 5
# baseline (speedup 1.0000x reference)
"""DistMult scoring kernel for Trainium2 (8 NeuronCores, SPMD).

score = sigmoid( (ent_emb[h] * diag(rel_emb[r])) @ ent_emb[t].T )
  batch_h/t/r: (2048,) int; ent_emb: (400000, 256) f32;
  rel_emb: (500, 256, 256) diagonal -> only its (500, 256) diagonal matters.

Strategy (all bf16 on device, fp32 accumulate in PSUM):
  - Heads sharded by GLOBAL SORTED order: core c computes score rows for the
    256 heads at sorted positions [256c, 256c+256). Tails are shared: every
    core uses all 2048 tails in GLOBAL SORTED order. The host un-permutes
    rows/columns when assembling the full (2048, 2048) output.
  - Tails: 16 dma_gather(transpose=True) window calls (128 sorted indices per
    call; each window spans <= 32767 rows so int16 offsets reach it). Window
    base addresses are compile-time AP offsets derived from the call's data
    (the kernel is JIT-specialized per window-base tuple and cached).
    dma_gather issues in ~90ns on gpsimd and runs async on 4 SWDGE queues,
    landing e-on-partition tiles directly - no PE transposes.
  - Heads: 2 x 128-row indirect DMA (int32 reach), then one SBUF-source
    dma_gather(transpose=True) to flip e onto partitions.
  - Rels: one dma_gather(transpose=True) from the 500-row diagonal table.
  - hrT = headsT * relsT on DVE; score matmuls on PE (bf16, k=256 over 2
    PSUM-accumulated 128-tiles); sigmoid on Scalar straight out of PSUM with
    bf16 output; HWDGE writes split across sync/vector queues.
  - A memset-fed dummy dma_gather absorbs the ~4.5us one-time Q7 library
    load; paced dummy matmuls keep the PE HAM clock at full speed before the
    real matmuls arrive.
"""

import sys

if "/opt/trn_rl_repo" not in sys.path:
    sys.path.insert(0, "/opt/trn_rl_repo")

import numpy as np
import ml_dtypes

import concourse.bass as bass
import concourse.tile as tile
from concourse import bacc, mybir

B = 2048
E = 256
N_ENT = 400000
N_REL = 500
CORES = 8
M = B // CORES  # 256 head rows per core
P = 128
WIN = 32768  # dma_gather int16 reach in table rows

F32 = mybir.dt.float32
BF16 = mybir.dt.bfloat16
I32 = mybir.dt.int32
I16 = mybir.dt.int16

NWARM_MM = 48  # PE HAM warmup matmuls


def build_nc(G, tail_bases):
    """G = number of 128-wide tail window buckets; tail_bases = their row bases."""
    nc = bacc.Bacc(
        "TRN2", target_bir_lowering=False, debug=False,
        num_devices=CORES, num_swdge_queues=1,
    )

    entb = nc.dram_tensor("entb", [N_ENT, E], BF16, kind="ExternalInput").ap()
    relb = nc.dram_tensor("relb", [512, E], BF16, kind="ExternalInput").ap()
    tidx = nc.dram_tensor("tidx", [P, 8 * G], I16, kind="ExternalInput").ap()
    hidx = nc.dram_tensor("hidx", [P, 2], I32, kind="ExternalInput").ap()
    ridx = nc.dram_tensor("ridx", [P, 16], I16, kind="ExternalInput").ap()
    iotah = nc.dram_tensor("iotah", [P, 16], I16, kind="ExternalInput").ap()
    score = nc.dram_tensor("score", [M, P * G], BF16, kind="ExternalOutput").ap()

    with tile.TileContext(nc) as tc:
        with (
            tc.tile_pool(name="idxp", bufs=1) as idx_pool,
            tc.tile_pool(name="gat", bufs=1) as gat_pool,
            tc.tile_pool(name="outp", bufs=8) as out_pool,
            tc.tile_pool(name="psmm", bufs=4, space="PSUM") as psum_mm,
            tc.tile_pool(name="pswm", bufs=1, space="PSUM") as psum_wm,
        ):
            # --- tiny inputs; warmup idx via memset (no DMA dependency) ---
            warmidx = idx_pool.tile([P, 1], I16)
            nc.vector.memset(warmidx[:], 0)
            wmt = gat_pool.tile([P, P], BF16, tag="wmt")
            nc.vector.memset(wmt[:], 0.0)

            hidx_sb = idx_pool.tile([P, 2], I32)
            nc.sync.dma_start(hidx_sb[:], hidx[:])
            tidx_sb = idx_pool.tile([P, 8 * G], I16)
            nc.sync.dma_start(tidx_sb[:], tidx[:])
            ridx_sb = idx_pool.tile([P, 16], I16)
            nc.sync.dma_start(ridx_sb[:], ridx[:])
            iotah_sb = idx_pool.tile([P, 16], I16)
            nc.sync.dma_start(iotah_sb[:], iotah[:])

            # --- Pool-engine DMA ops: tile assigns DMASW sems round-robin
            # over 8 lanes in emission order; with 4 SWDGE queues, sem lane s
            # must always pair with queue s%4, so every Pool DMA here is
            # emitted at rotation position p with queue_num=p%4, and the
            # queue-0-only indirect DMAs sit at positions 0 mod 4. ---

            # pos 0: warmup gather (q0) - absorbs one-time Q7 mlp lib load
            wjunk = gat_pool.tile([P, 1, E], BF16, tag="wjunk")
            nc.gpsimd.dma_gather(
                out_ap=wjunk[:], in_ap=relb[:], idxs_ap=warmidx[:],
                num_idxs=16, num_idxs_reg=16, elem_size=E,
                transpose=False, queue_num=0,
            )

            # pos 1: rels transposed gather (q1)
            rT = gat_pool.tile([P, 2, M], BF16, tag="rT")
            nc.gpsimd.dma_gather(
                out_ap=rT[:], in_ap=relb[:], idxs_ap=ridx_sb[:],
                num_idxs=M, num_idxs_reg=M, elem_size=E,
                transpose=True, queue_num=0,
            )

            hrows = gat_pool.tile([P, 2, E], BF16, tag="hrows")
            tbig = gat_pool.tile([P, G, 2, P], BF16, tag="tbig")

            def tail_gather(w, q):
                base = tail_bases[w]
                nc.gpsimd.dma_gather(
                    out_ap=tbig[:, w, :, :],
                    in_ap=entb[base : base + WIN, :],
                    idxs_ap=tidx_sb[:, 8 * w : 8 * w + 8],
                    num_idxs=P, num_idxs_reg=P, elem_size=E,
                    transpose=True, queue_num=0,
                )

            def head_gather(j):
                nc.gpsimd.indirect_dma_start(
                    out=hrows[:, j, :],
                    out_offset=None,
                    in_=entb[:],
                    in_offset=bass.IndirectOffsetOnAxis(
                        ap=hidx_sb[:, j : j + 1], axis=0
                    ),
                )

            # pos 2..19: tails on rotating queues; heads indirect at pos 4, 8
            pos = 2
            w = 0
            plan = []
            heads_left = 2
            while w < G or heads_left:
                if heads_left and pos % 4 == 0:
                    head_gather(2 - heads_left)
                    heads_left -= 1
                elif w < G:
                    tail_gather(w, pos % 4)
                    w += 1
                else:
                    break
                pos += 1
            assert heads_left == 0 and w == G, (heads_left, w)

            # final pos: heads transpose via SBUF-source gather
            hT = gat_pool.tile([P, 2, M], BF16, tag="hT")
            nc.gpsimd.dma_gather(
                out_ap=hT[:], in_ap=hrows[:], idxs_ap=iotah_sb[:],
                num_idxs=M, num_idxs_reg=M, elem_size=E,
                transpose=True, queue_num=0,
                sbuf_tokens_per_rank=P, sbuf_free_dim_per_rank=512,
                sbuf_free_dim_pad_per_rank=0, sbuf_byte_offset=0,
            )

            # --- PE HAM warmup: paced dummy matmuls on a zero tile ---
            wpsum = psum_wm.tile([P, P], F32)
            for _ in range(NWARM_MM):
                nc.tensor.matmul(wpsum[:], lhsT=wmt[:], rhs=wmt[:], start=True, stop=True)

            # --- hrT = hT * rT (DVE) ---
            hrT = gat_pool.tile([P, 2, M], BF16, tag="hrT")
            nc.vector.tensor_mul(hrT[:], hT[:], rT[:])

            # --- score matmuls + sigmoid + out, n-chunks of 512 cols ---
            n_chunks = (G + 3) // 4
            for c in range(n_chunks):
                w0 = 4 * c
                wn = min(4, G - w0)
                ncols = wn * P
                for i in range(M // P):
                    ps = psum_mm.tile([P, ncols], F32, tag="ps", name=f"ps{c}_{i}")
                    for k in range(2):
                        nc.tensor.matmul(
                            ps[:],
                            lhsT=hrT[:, k, i * P : (i + 1) * P],
                            rhs=tbig[:, w0 : w0 + wn, k, :],
                            start=(k == 0),
                            stop=(k == 1),
                        )
                    ob = out_pool.tile([P, ncols], BF16, tag="ob", name=f"ob{c}_{i}")
                    nc.scalar.activation(
                        ob[:], ps[:], mybir.ActivationFunctionType.Sigmoid
                    )
                    nc.sync.dma_start(
                        score[i * P : (i + 1) * P, w0 * P : w0 * P + ncols], ob[:]
                    )

    nc.compile()
    return nc


_NC_CACHE = {}


def _get_nc(G, tail_bases):
    key = (G, tuple(tail_bases))
    if key not in _NC_CACHE:
        _NC_CACHE[key] = build_nc(G, tail_bases)
    return _NC_CACHE[key]


def _wrap16(idx, reps=8):
    """Position i of a gather call reads idxs[i % 16, i // 16]; replicate to 128 rows."""
    n = idx.shape[0]
    w = idx.reshape(n // 16, 16).T
    return np.ascontiguousarray(np.tile(w, (reps, 1)))


def _plan_tail_buckets(bt_sorted):
    """Greedy exact-128 buckets of sorted tail indices; pad a bucket (repeating
    its first index) when 128 consecutive sorted values span > 32767 rows.
    Returns (bases, lo_idx [G*128] int16, keep [G*128] bool)."""
    n = bt_sorted.shape[0]
    bases, lo_all, keep_all = [], [], []
    pos = 0
    while pos < n:
        chunk = bt_sorted[pos : pos + P]
        span = int(chunk[-1]) - int(chunk[0])
        if span <= WIN - 1:
            take = len(chunk)
        else:
            take = int(np.searchsorted(chunk, chunk[0] + WIN, side="left"))
        vals = chunk[:take]
        pad = P - take
        if pad:
            vals = np.concatenate([vals, np.full(pad, vals[0], dtype=vals.dtype)])
        base = min(int(vals.min()), N_ENT - WIN)
        bases.append(base)
        lo_all.append((vals - base).astype(np.int16))
        keep_all.append(np.arange(P) < take)
        pos += take
    return bases, np.concatenate(lo_all), np.concatenate(keep_all)


def prepare(batch_h, batch_t, batch_r, ent_emb, rel_emb):
    bh = np.asarray(batch_h).astype(np.int64)
    bt = np.asarray(batch_t).astype(np.int64)
    br = np.asarray(batch_r).astype(np.int64)

    entb = np.asarray(ent_emb).astype(ml_dtypes.bfloat16)
    rel_np = np.asarray(rel_emb)
    rel_diag = rel_np[:, np.arange(E), np.arange(E)].astype(ml_dtypes.bfloat16)
    relb = np.zeros((512, E), dtype=ml_dtypes.bfloat16)
    relb[:N_REL] = rel_diag

    # tails: global sort -> window buckets (shared by all cores)
    t_order = np.argsort(bt, kind="stable")
    bases, t_lo, t_keep = _plan_tail_buckets(bt[t_order])
    G = len(bases)
    tidx = _wrap16(t_lo)  # [128, 8G]

    # heads: global sort -> per-core slices of 256
    h_order = np.argsort(bh, kind="stable")
    iotah = _wrap16(np.arange(M, dtype=np.int16))

    in_maps = []
    for c in range(CORES):
        rows = h_order[c * M : (c + 1) * M]  # original batch positions
        hvals = bh[rows].astype(np.int32)
        hidx = np.ascontiguousarray(hvals.reshape(2, P).T)  # [128, 2] col j = rows 128j+p
        rvals = br[rows].astype(np.int16)
        ridx = _wrap16(rvals)
        in_maps.append(
            {
                "entb": entb, "relb": relb, "tidx": tidx, "hidx": hidx,
                "ridx": ridx, "iotah": iotah,
            }
        )
    meta = {
        "G": G, "bases": tuple(int(b) for b in bases),
        "t_order": t_order, "t_keep": t_keep, "h_order": h_order,
    }
    return in_maps, meta


def run(batch_h, batch_t, batch_r, ent_emb, rel_emb, trace=False, tmpdir=None):
    from concourse.bass_utils import run_bass_kernel_spmd

    in_maps, meta = prepare(batch_h, batch_t, batch_r, ent_emb, rel_emb)
    nc = _get_nc(meta["G"], meta["bases"])
    kwargs = {}
    if trace:
        kwargs = {"trace": True, "tmpdir": tmpdir}
    res = run_bass_kernel_spmd(nc, in_maps, core_ids=list(range(CORES)), **kwargs)

    G = meta["G"]
    keep = meta["t_keep"]
    col_src = np.nonzero(keep)[0]  # device cols holding real sorted positions
    t_cols = meta["t_order"]  # sorted position -> original batch column
    full = np.empty((B, B), dtype=np.float32)
    for c in range(CORES):
        blk = np.asarray(res.results[c]["score"])  # [256, 128G] bf16
        rows = meta["h_order"][c * M : (c + 1) * M]
        full[np.ix_(rows, t_cols)] = blk[:, col_src].astype(np.float32)
    return full, res


def kernel(batch_h, batch_t, batch_r, ent_emb, rel_emb):
    score, _ = run(batch_h, batch_t, batch_r, ent_emb, rel_emb)
    return score


# revision 6
# speedup vs baseline: 1.3189x; 1.3189x over previous
"""DistMult scoring kernel for Trainium2 (8 NeuronCores, SPMD).

score = sigmoid( (ent_emb[h] * diag(rel_emb[r])) @ ent_emb[t].T )
  batch_h/t/r: (2048,) int; ent_emb: (400000, 256) f32;
  rel_emb: (500, 256, 256) diagonal -> only its (500, 256) diagonal matters.

Strategy (all bf16 on device, fp32 accumulate in PSUM):
  - Heads sharded by GLOBAL SORTED order: core c computes score rows for the
    256 heads at sorted positions [256c, 256c+256). Tails are shared: every
    core uses all 2048 tails in GLOBAL SORTED order. The host un-permutes
    rows/columns when assembling the full (2048, 2048) output.
  - Tails: 16 dma_gather(transpose=True) window calls (128 sorted indices per
    call; each window spans <= 32767 rows so int16 offsets reach it). Window
    base addresses are compile-time AP offsets derived from the call's data
    (the kernel is JIT-specialized per window-base tuple and cached).
    dma_gather issues in ~90ns on gpsimd and runs async on 4 SWDGE queues,
    landing e-on-partition tiles directly - no PE transposes.
  - Heads: 2 x 128-row indirect DMA (int32 reach), then one SBUF-source
    dma_gather(transpose=True) to flip e onto partitions.
  - Rels: one dma_gather(transpose=True) from the 500-row diagonal table.
  - hrT = headsT * relsT on DVE; score matmuls on PE (bf16, k=256 over 2
    PSUM-accumulated 128-tiles); sigmoid on Scalar straight out of PSUM with
    bf16 output; HWDGE writes split across sync/vector queues.
  - A memset-fed dummy dma_gather absorbs the ~4.5us one-time Q7 library
    load; paced dummy matmuls keep the PE HAM clock at full speed before the
    real matmuls arrive.
"""

import sys

if "/opt/trn_rl_repo" not in sys.path:
    sys.path.insert(0, "/opt/trn_rl_repo")

import numpy as np
import ml_dtypes

import concourse.bass as bass
import concourse.tile as tile
from concourse import bacc, mybir

B = 2048
E = 256
N_ENT = 400000
N_REL = 500
CORES = 8
M = B // CORES  # 256 head rows per core
P = 128
WIN = 32768  # dma_gather int16 reach in table rows

F32 = mybir.dt.float32
BF16 = mybir.dt.bfloat16
I32 = mybir.dt.int32
I16 = mybir.dt.int16

NWARM_MM = 48  # PE HAM warmup matmuls


def build_nc(G, tail_bases):
    """G = number of 128-wide tail window buckets; tail_bases = their row bases."""
    nc = bacc.Bacc(
        "TRN2", target_bir_lowering=False, debug=False,
        num_devices=CORES, num_swdge_queues=4,
    )

    entb = nc.dram_tensor("entb", [N_ENT, E], BF16, kind="ExternalInput").ap()
    relb = nc.dram_tensor("relb", [512, E], BF16, kind="ExternalInput").ap()
    tidx = nc.dram_tensor("tidx", [P, 8 * G], I16, kind="ExternalInput").ap()
    hidx = nc.dram_tensor("hidx", [P, 2], I32, kind="ExternalInput").ap()
    ridx = nc.dram_tensor("ridx", [P, 16], I16, kind="ExternalInput").ap()
    iotah = nc.dram_tensor("iotah", [P, 16], I16, kind="ExternalInput").ap()
    score = nc.dram_tensor("score", [M, P * G], BF16, kind="ExternalOutput").ap()

    with tile.TileContext(nc) as tc:
        with (
            tc.tile_pool(name="idxp", bufs=1) as idx_pool,
            tc.tile_pool(name="gat", bufs=1) as gat_pool,
            tc.tile_pool(name="outp", bufs=8) as out_pool,
            tc.tile_pool(name="psmm", bufs=4, space="PSUM") as psum_mm,
            tc.tile_pool(name="pswm", bufs=1, space="PSUM") as psum_wm,
        ):
            # --- tiny inputs; warmup idx via memset (no DMA dependency) ---
            warmidx = idx_pool.tile([P, 1], I16)
            nc.vector.memset(warmidx[:], 0)
            wmt = gat_pool.tile([P, P], BF16, tag="wmt")
            nc.vector.memset(wmt[:], 0.0)

            hidx_sb = idx_pool.tile([P, 2], I32)
            nc.sync.dma_start(hidx_sb[:], hidx[:])
            tidx_sb = idx_pool.tile([P, 8 * G], I16)
            nc.sync.dma_start(tidx_sb[:], tidx[:])
            ridx_sb = idx_pool.tile([P, 16], I16)
            nc.sync.dma_start(ridx_sb[:], ridx[:])
            iotah_sb = idx_pool.tile([P, 16], I16)
            nc.sync.dma_start(iotah_sb[:], iotah[:])

            # --- Pool-engine DMA ops: tile assigns DMASW sems round-robin
            # over 8 lanes in emission order; with 4 SWDGE queues, sem lane s
            # must always pair with queue s%4, so every Pool DMA here is
            # emitted at rotation position p with queue_num=p%4, and the
            # queue-0-only indirect DMAs sit at positions 0 mod 4. ---

            # pos 0: warmup gather (q0) - absorbs one-time Q7 mlp lib load
            wjunk = gat_pool.tile([P, 1, E], BF16, tag="wjunk")
            nc.gpsimd.dma_gather(
                out_ap=wjunk[:], in_ap=relb[:], idxs_ap=warmidx[:],
                num_idxs=16, num_idxs_reg=16, elem_size=E,
                transpose=False, queue_num=0,
            )

            # pos 1: rels transposed gather (q1)
            rT = gat_pool.tile([P, 2, M], BF16, tag="rT")
            nc.gpsimd.dma_gather(
                out_ap=rT[:], in_ap=relb[:], idxs_ap=ridx_sb[:],
                num_idxs=M, num_idxs_reg=M, elem_size=E,
                transpose=True, queue_num=0,
            )

            hrows = gat_pool.tile([P, 2, E], BF16, tag="hrows")
            tbig = gat_pool.tile([P, G, 2, P], BF16, tag="tbig")

            def tail_gather(w, q):
                base = tail_bases[w]
                nc.gpsimd.dma_gather(
                    out_ap=tbig[:, w, :, :],
                    in_ap=entb[base : base + WIN, :],
                    idxs_ap=tidx_sb[:, 8 * w : 8 * w + 8],
                    num_idxs=P, num_idxs_reg=P, elem_size=E,
                    transpose=True, queue_num=0,
                )

            def head_gather(j):
                nc.gpsimd.indirect_dma_start(
                    out=hrows[:, j, :],
                    out_offset=None,
                    in_=entb[:],
                    in_offset=bass.IndirectOffsetOnAxis(
                        ap=hidx_sb[:, j : j + 1], axis=0
                    ),
                )

            # emission order: heads indirect, first tails, heads transpose
            # early (it gates hrT and all matmuls), remaining tails.
            head_gather(0)
            head_gather(1)
            for w in range(4):
                tail_gather(w, 0)

            hT = gat_pool.tile([P, 2, M], BF16, tag="hT")
            nc.gpsimd.dma_gather(
                out_ap=hT[:], in_ap=hrows[:], idxs_ap=iotah_sb[:],
                num_idxs=M, num_idxs_reg=M, elem_size=E,
                transpose=True, queue_num=0,
                sbuf_tokens_per_rank=P, sbuf_free_dim_per_rank=512,
                sbuf_free_dim_pad_per_rank=0, sbuf_byte_offset=0,
            )
            for w in range(4, G):
                tail_gather(w, 0)

            # --- PE HAM warmup: a small upfront batch on the zero tile,
            # then 2 dummy matmuls per landed tail bucket so PE activity is
            # paced across the whole gather phase (keeps HAM at full clock
            # right up to the real matmuls) ---
            wpsum = psum_wm.tile([P, P], F32)
            for _ in range(12):
                nc.tensor.matmul(wpsum[:], lhsT=wmt[:], rhs=wmt[:], start=True, stop=True)
            for w in range(G):
                t = tbig[:, w, 0, :]
                for _ in range(2):
                    nc.tensor.matmul(wpsum[:], lhsT=t, rhs=t, start=True, stop=True)

            # --- hrT = hT * rT (DVE) ---
            hrT = gat_pool.tile([P, 2, M], BF16, tag="hrT")
            nc.vector.tensor_mul(hrT[:], hT[:], rT[:])

            # --- score matmuls + sigmoid + out, n-chunks of 512 cols ---
            n_chunks = (G + 3) // 4
            for c in range(n_chunks):
                w0 = 4 * c
                wn = min(4, G - w0)
                ncols = wn * P
                for i in range(M // P):
                    ps = psum_mm.tile([P, ncols], F32, tag="ps", name=f"ps{c}_{i}")
                    for k in range(2):
                        nc.tensor.matmul(
                            ps[:],
                            lhsT=hrT[:, k, i * P : (i + 1) * P],
                            rhs=tbig[:, w0 : w0 + wn, k, :],
                            start=(k == 0),
                            stop=(k == 1),
                        )
                    ob = out_pool.tile([P, ncols], BF16, tag="ob", name=f"ob{c}_{i}")
                    nc.scalar.activation(
                        ob[:], ps[:], mybir.ActivationFunctionType.Sigmoid
                    )
                    nc.sync.dma_start(
                        score[i * P : (i + 1) * P, w0 * P : w0 * P + ncols], ob[:]
                    )

    # Tile assigns DMASW completion sems round-robin over 8 lanes in
    # scheduled order, and the SWDGE shadow-sem bookkeeping requires each
    # sem to be driven by exactly one queue. Rewrite each gather's queue to
    # a pure function of its assigned lane (lanes used by the queue-0-only
    # indirect DMAs stay on queue 0).
    import re as _re

    pool_dmas = []
    for bb in nc.main_func.blocks:
        for inst in bb.instructions:
            if inst.engine != mybir.EngineType.Pool:
                continue
            si = inst.sync_info
            if not si or not si.on_update:
                continue
            m = _re.match(r"DMASW(\d+)_", si.on_update[0].ant_name or "")
            if not m:
                continue
            pool_dmas.append((inst, int(m.group(1))))
    indirect_lanes = {
        lane for inst, lane in pool_dmas if isinstance(inst, mybir.InstDMACopy)
    }
    for inst, lane in pool_dmas:
        if isinstance(inst, mybir.InstDMAGatherAnt):
            inst.queue_num = 0 if lane in indirect_lanes else lane % 4

    nc.compile()
    return nc


_NC_CACHE = {}


def _get_nc(G, tail_bases):
    key = (G, tuple(tail_bases))
    if key not in _NC_CACHE:
        _NC_CACHE[key] = build_nc(G, tail_bases)
    return _NC_CACHE[key]


def _wrap16(idx, reps=8):
    """Position i of a gather call reads idxs[i % 16, i // 16]; replicate to 128 rows."""
    n = idx.shape[0]
    w = idx.reshape(n // 16, 16).T
    return np.ascontiguousarray(np.tile(w, (reps, 1)))


def _plan_tail_buckets(bt_sorted):
    """Greedy exact-128 buckets of sorted tail indices; pad a bucket (repeating
    its first index) when 128 consecutive sorted values span > 32767 rows.
    Returns (bases, lo_idx [G*128] int16, keep [G*128] bool)."""
    n = bt_sorted.shape[0]
    bases, lo_all, keep_all = [], [], []
    pos = 0
    while pos < n:
        chunk = bt_sorted[pos : pos + P]
        span = int(chunk[-1]) - int(chunk[0])
        if span <= WIN - 1:
            take = len(chunk)
        else:
            take = int(np.searchsorted(chunk, chunk[0] + WIN, side="left"))
        vals = chunk[:take]
        pad = P - take
        if pad:
            vals = np.concatenate([vals, np.full(pad, vals[0], dtype=vals.dtype)])
        base = min(int(vals.min()), N_ENT - WIN)
        bases.append(base)
        lo_all.append((vals - base).astype(np.int16))
        keep_all.append(np.arange(P) < take)
        pos += take
    return bases, np.concatenate(lo_all), np.concatenate(keep_all)


def prepare(batch_h, batch_t, batch_r, ent_emb, rel_emb):
    bh = np.asarray(batch_h).astype(np.int64)
    bt = np.asarray(batch_t).astype(np.int64)
    br = np.asarray(batch_r).astype(np.int64)

    entb = np.asarray(ent_emb).astype(ml_dtypes.bfloat16)
    rel_np = np.asarray(rel_emb)
    rel_diag = rel_np[:, np.arange(E), np.arange(E)].astype(ml_dtypes.bfloat16)
    relb = np.zeros((512, E), dtype=ml_dtypes.bfloat16)
    relb[:N_REL] = rel_diag

    # tails: global sort -> window buckets (shared by all cores)
    t_order = np.argsort(bt, kind="stable")
    bases, t_lo, t_keep = _plan_tail_buckets(bt[t_order])
    G = len(bases)
    tidx = _wrap16(t_lo)  # [128, 8G]

    # heads: global sort -> per-core slices of 256
    h_order = np.argsort(bh, kind="stable")
    iotah = _wrap16(np.arange(M, dtype=np.int16))

    in_maps = []
    for c in range(CORES):
        rows = h_order[c * M : (c + 1) * M]  # original batch positions
        hvals = bh[rows].astype(np.int32)
        hidx = np.ascontiguousarray(hvals.reshape(2, P).T)  # [128, 2] col j = rows 128j+p
        rvals = br[rows].astype(np.int16)
        ridx = _wrap16(rvals)
        in_maps.append(
            {
                "entb": entb, "relb": relb, "tidx": tidx, "hidx": hidx,
                "ridx": ridx, "iotah": iotah,
            }
        )
    meta = {
        "G": G, "bases": tuple(int(b) for b in bases),
        "t_order": t_order, "t_keep": t_keep, "h_order": h_order,
    }
    return in_maps, meta


def run(batch_h, batch_t, batch_r, ent_emb, rel_emb, trace=False, tmpdir=None):
    from concourse.bass_utils import run_bass_kernel_spmd

    in_maps, meta = prepare(batch_h, batch_t, batch_r, ent_emb, rel_emb)
    nc = _get_nc(meta["G"], meta["bases"])
    kwargs = {}
    if trace:
        kwargs = {"trace": True, "tmpdir": tmpdir}
    res = run_bass_kernel_spmd(nc, in_maps, core_ids=list(range(CORES)), **kwargs)

    G = meta["G"]
    keep = meta["t_keep"]
    col_src = np.nonzero(keep)[0]  # device cols holding real sorted positions
    t_cols = meta["t_order"]  # sorted position -> original batch column
    full = np.empty((B, B), dtype=np.float32)
    for c in range(CORES):
        blk = np.asarray(res.results[c]["score"])  # [256, 128G] bf16
        rows = meta["h_order"][c * M : (c + 1) * M]
        full[np.ix_(rows, t_cols)] = blk[:, col_src].astype(np.float32)
    return full, res


def kernel(batch_h, batch_t, batch_r, ent_emb, rel_emb):
    score, _ = run(batch_h, batch_t, batch_r, ent_emb, rel_emb)
    return score


# revision 7
# speedup vs baseline: 1.6172x; 1.2262x over previous
"""DistMult scoring kernel for Trainium2 (8 NeuronCores, SPMD).

score = sigmoid( (ent_emb[h] * diag(rel_emb[r])) @ ent_emb[t].T )
  batch_h/t/r: (2048,) int; ent_emb: (400000, 256) f32;
  rel_emb: (500, 256, 256) diagonal -> only its (500, 256) diagonal matters.

Strategy (all bf16 on device, fp32 accumulate in PSUM):
  - Heads sharded by GLOBAL SORTED order: core c computes score rows for the
    256 heads at sorted positions [256c, 256c+256). Tails are shared: every
    core uses all 2048 tails in GLOBAL SORTED order. The host un-permutes
    rows/columns when assembling the full (2048, 2048) output.
  - Tails: 16 dma_gather(transpose=True) window calls (128 sorted indices per
    call; each window spans <= 32767 rows so int16 offsets reach it). Window
    base addresses are compile-time AP offsets derived from the call's data
    (the kernel is JIT-specialized per window-base tuple and cached).
    dma_gather issues in ~90ns on gpsimd and runs async on 4 SWDGE queues,
    landing e-on-partition tiles directly - no PE transposes.
  - Heads: 2 x 128-row indirect DMA (int32 reach), then one SBUF-source
    dma_gather(transpose=True) to flip e onto partitions.
  - Rels: one dma_gather(transpose=True) from the 500-row diagonal table.
  - hrT = headsT * relsT on DVE; score matmuls on PE (bf16, k=256 over 2
    PSUM-accumulated 128-tiles); sigmoid on Scalar straight out of PSUM with
    bf16 output; HWDGE writes split across sync/vector queues.
  - A memset-fed dummy dma_gather absorbs the ~4.5us one-time Q7 library
    load; paced dummy matmuls keep the PE HAM clock at full speed before the
    real matmuls arrive.
"""

import sys

if "/opt/trn_rl_repo" not in sys.path:
    sys.path.insert(0, "/opt/trn_rl_repo")

import numpy as np
import ml_dtypes

import concourse.bass as bass
import concourse.tile as tile
from concourse import bacc, mybir


def _mlp_lib():
    from concourse.library_config import mlp

    return mlp

B = 2048
E = 256
N_ENT = 400000
N_REL = 500
CORES = 8
M = B // CORES  # 256 head rows per core
P = 128
WIN = 32768  # dma_gather int16 reach in table rows

F32 = mybir.dt.float32
BF16 = mybir.dt.bfloat16
I32 = mybir.dt.int32
I16 = mybir.dt.int16

NWARM_MM = 48  # PE HAM warmup matmuls


def build_nc(G, tail_bases):
    """G = number of 128-wide tail window buckets; tail_bases = their row bases."""
    nc = bacc.Bacc(
        "TRN2", target_bir_lowering=False, debug=False,
        num_devices=CORES, num_swdge_queues=4,
    )

    entb = nc.dram_tensor("entb", [N_ENT, E], BF16, kind="ExternalInput").ap()
    relb = nc.dram_tensor("relb", [512, E], BF16, kind="ExternalInput").ap()
    tidx = nc.dram_tensor("tidx", [P, 8 * G], I16, kind="ExternalInput").ap()
    hidx = nc.dram_tensor("hidx", [P, 2], I32, kind="ExternalInput").ap()
    ridx = nc.dram_tensor("ridx", [P, 16], I16, kind="ExternalInput").ap()
    iotah = nc.dram_tensor("iotah", [P, 16], I16, kind="ExternalInput").ap()
    score = nc.dram_tensor("score", [M, P * G], BF16, kind="ExternalOutput").ap()

    with tile.TileContext(nc) as tc:
        with (
            tc.tile_pool(name="idxp", bufs=1) as idx_pool,
            tc.tile_pool(name="gat", bufs=1) as gat_pool,
            tc.tile_pool(name="outp", bufs=8) as out_pool,
            tc.tile_pool(name="psmm", bufs=4, space="PSUM") as psum_mm,
            tc.tile_pool(name="pswm", bufs=1, space="PSUM") as psum_wm,
        ):
            # --- explicit early library load; zero tile for PE warmup ---
            nc.gpsimd.load_library(_mlp_lib())
            wmt = gat_pool.tile([P, P], BF16, tag="wmt")
            nc.vector.memset(wmt[:], 0.0)

            hidx_sb = idx_pool.tile([P, 2], I32)
            nc.sync.dma_start(hidx_sb[:], hidx[:])
            tidx_sb = idx_pool.tile([P, 8 * G], I16)
            nc.sync.dma_start(tidx_sb[:], tidx[:])
            ridx_sb = idx_pool.tile([P, 16], I16)
            nc.sync.dma_start(ridx_sb[:], ridx[:])
            iotah_sb = idx_pool.tile([P, 16], I16)
            nc.sync.dma_start(iotah_sb[:], iotah[:])

            # --- Pool-engine DMA ops (queues rewritten post-schedule) ---

            # rels transposed gather - first mlp op, pays the post-load sync
            rT = gat_pool.tile([P, 2, M], BF16, tag="rT")
            nc.gpsimd.dma_gather(
                out_ap=rT[:], in_ap=relb[:], idxs_ap=ridx_sb[:],
                num_idxs=M, num_idxs_reg=M, elem_size=E,
                transpose=True, queue_num=0,
            )

            hrows = gat_pool.tile([P, 2, E], BF16, tag="hrows")
            tbig = gat_pool.tile([P, G, 2, P], BF16, tag="tbig")

            def tail_gather(w, q):
                base = tail_bases[w]
                nc.gpsimd.dma_gather(
                    out_ap=tbig[:, w, :, :],
                    in_ap=entb[base : base + WIN, :],
                    idxs_ap=tidx_sb[:, 8 * w : 8 * w + 8],
                    num_idxs=P, num_idxs_reg=P, elem_size=E,
                    transpose=True, queue_num=0,
                )

            def head_gather(j):
                nc.gpsimd.indirect_dma_start(
                    out=hrows[:, j, :],
                    out_offset=None,
                    in_=entb[:],
                    in_offset=bass.IndirectOffsetOnAxis(
                        ap=hidx_sb[:, j : j + 1], axis=0
                    ),
                )

            # emission order: heads indirect, first tails, heads transpose
            # early (it gates hrT and all matmuls), remaining tails.
            head_gather(0)
            head_gather(1)
            for w in range(4):
                tail_gather(w, 0)

            hT = gat_pool.tile([P, 2, M], BF16, tag="hT")
            nc.gpsimd.dma_gather(
                out_ap=hT[:], in_ap=hrows[:], idxs_ap=iotah_sb[:],
                num_idxs=M, num_idxs_reg=M, elem_size=E,
                transpose=True, queue_num=0,
                sbuf_tokens_per_rank=P, sbuf_free_dim_per_rank=512,
                sbuf_free_dim_pad_per_rank=0, sbuf_byte_offset=0,
            )
            for w in range(4, G):
                tail_gather(w, 0)

            # --- PE HAM warmup: a few early dummies, then a dense block
            # gated on the rels tile landing (~just before real matmuls) so
            # the clock is at full speed when the score matmuls start ---
            wpsum = psum_wm.tile([P, P], F32)
            for _ in range(8):
                nc.tensor.matmul(wpsum[:], lhsT=wmt[:], rhs=wmt[:], start=True, stop=True)
            rwarm = rT[:, 0, 0:P]
            for _ in range(17):
                nc.tensor.matmul(wpsum[:], lhsT=rwarm, rhs=rwarm, start=True, stop=True)

            # --- hrT = hT * rT (DVE) ---
            hrT = gat_pool.tile([P, 2, M], BF16, tag="hrT")
            nc.vector.tensor_mul(hrT[:], hT[:], rT[:])

            # --- score matmuls + sigmoid + out, n-chunks of 512 cols ---
            n_chunks = (G + 3) // 4
            for c in range(n_chunks):
                w0 = 4 * c
                wn = min(4, G - w0)
                ncols = wn * P
                for i in range(M // P):
                    ps = psum_mm.tile([P, ncols], F32, tag="ps", name=f"ps{c}_{i}")
                    for k in range(2):
                        nc.tensor.matmul(
                            ps[:],
                            lhsT=hrT[:, k, i * P : (i + 1) * P],
                            rhs=tbig[:, w0 : w0 + wn, k, :],
                            start=(k == 0),
                            stop=(k == 1),
                        )
                    ob = out_pool.tile([P, ncols], BF16, tag="ob", name=f"ob{c}_{i}")
                    nc.scalar.activation(
                        ob[:], ps[:], mybir.ActivationFunctionType.Sigmoid
                    )
                    nc.sync.dma_start(
                        score[i * P : (i + 1) * P, w0 * P : w0 * P + ncols], ob[:]
                    )

    # Tile assigns DMASW completion sems round-robin over 8 lanes in
    # scheduled order, and the SWDGE shadow-sem bookkeeping requires each
    # sem to be driven by exactly one queue. Rewrite each gather's queue to
    # a pure function of its assigned lane (lanes used by the queue-0-only
    # indirect DMAs stay on queue 0).
    import re as _re

    pool_dmas = []
    for bb in nc.main_func.blocks:
        for inst in bb.instructions:
            if inst.engine != mybir.EngineType.Pool:
                continue
            si = inst.sync_info
            if not si or not si.on_update:
                continue
            m = _re.match(r"DMASW(\d+)_", si.on_update[0].ant_name or "")
            if not m:
                continue
            pool_dmas.append((inst, int(m.group(1))))
    indirect_lanes = {
        lane for inst, lane in pool_dmas if isinstance(inst, mybir.InstDMACopy)
    }
    qmap = {lane: 0 for lane in indirect_lanes}
    free_lanes = [ln for ln in range(8) if ln not in indirect_lanes]
    fill = ([1, 2, 3] if indirect_lanes else [0, 1, 2, 3]) * 8
    for i, ln in enumerate(free_lanes):
        qmap[ln] = fill[i]
    for inst, lane in pool_dmas:
        if isinstance(inst, mybir.InstDMAGatherAnt):
            inst.queue_num = qmap[lane]

    nc.compile()
    return nc


_NC_CACHE = {}


def _get_nc(G, tail_bases):
    key = (G, tuple(tail_bases))
    if key not in _NC_CACHE:
        _NC_CACHE[key] = build_nc(G, tail_bases)
    return _NC_CACHE[key]


def _wrap16(idx, reps=8):
    """Position i of a gather call reads idxs[i % 16, i // 16]; replicate to 128 rows."""
    n = idx.shape[0]
    w = idx.reshape(n // 16, 16).T
    return np.ascontiguousarray(np.tile(w, (reps, 1)))


def _plan_tail_buckets(bt_sorted):
    """Greedy exact-128 buckets of sorted tail indices; pad a bucket (repeating
    its first index) when 128 consecutive sorted values span > 32767 rows.
    Returns (bases, lo_idx [G*128] int16, keep [G*128] bool)."""
    n = bt_sorted.shape[0]
    bases, lo_all, keep_all = [], [], []
    pos = 0
    while pos < n:
        chunk = bt_sorted[pos : pos + P]
        span = int(chunk[-1]) - int(chunk[0])
        if span <= WIN - 1:
            take = len(chunk)
        else:
            take = int(np.searchsorted(chunk, chunk[0] + WIN, side="left"))
        vals = chunk[:take]
        pad = P - take
        if pad:
            vals = np.concatenate([vals, np.full(pad, vals[0], dtype=vals.dtype)])
        base = min(int(vals.min()), N_ENT - WIN)
        bases.append(base)
        lo_all.append((vals - base).astype(np.int16))
        keep_all.append(np.arange(P) < take)
        pos += take
    return bases, np.concatenate(lo_all), np.concatenate(keep_all)


def prepare(batch_h, batch_t, batch_r, ent_emb, rel_emb):
    bh = np.asarray(batch_h).astype(np.int64)
    bt = np.asarray(batch_t).astype(np.int64)
    br = np.asarray(batch_r).astype(np.int64)

    entb = np.asarray(ent_emb).astype(ml_dtypes.bfloat16)
    rel_np = np.asarray(rel_emb)
    rel_diag = rel_np[:, np.arange(E), np.arange(E)].astype(ml_dtypes.bfloat16)
    relb = np.zeros((512, E), dtype=ml_dtypes.bfloat16)
    relb[:N_REL] = rel_diag

    # tails: global sort -> window buckets (shared by all cores)
    t_order = np.argsort(bt, kind="stable")
    bases, t_lo, t_keep = _plan_tail_buckets(bt[t_order])
    G = len(bases)
    tidx = _wrap16(t_lo)  # [128, 8G]

    # heads: global sort -> per-core slices of 256
    h_order = np.argsort(bh, kind="stable")
    iotah = _wrap16(np.arange(M, dtype=np.int16))

    in_maps = []
    for c in range(CORES):
        rows = h_order[c * M : (c + 1) * M]  # original batch positions
        hvals = bh[rows].astype(np.int32)
        hidx = np.ascontiguousarray(hvals.reshape(2, P).T)  # [128, 2] col j = rows 128j+p
        rvals = br[rows].astype(np.int16)
        ridx = _wrap16(rvals)
        in_maps.append(
            {
                "entb": entb, "relb": relb, "tidx": tidx, "hidx": hidx,
                "ridx": ridx, "iotah": iotah,
            }
        )
    meta = {
        "G": G, "bases": tuple(int(b) for b in bases),
        "t_order": t_order, "t_keep": t_keep, "h_order": h_order,
    }
    return in_maps, meta


def run(batch_h, batch_t, batch_r, ent_emb, rel_emb, trace=False, tmpdir=None):
    from concourse.bass_utils import run_bass_kernel_spmd

    in_maps, meta = prepare(batch_h, batch_t, batch_r, ent_emb, rel_emb)
    nc = _get_nc(meta["G"], meta["bases"])
    kwargs = {}
    if trace:
        kwargs = {"trace": True, "tmpdir": tmpdir}
    res = run_bass_kernel_spmd(nc, in_maps, core_ids=list(range(CORES)), **kwargs)

    G = meta["G"]
    keep = meta["t_keep"]
    col_src = np.nonzero(keep)[0]  # device cols holding real sorted positions
    t_cols = meta["t_order"]  # sorted position -> original batch column
    full = np.empty((B, B), dtype=np.float32)
    for c in range(CORES):
        blk = np.asarray(res.results[c]["score"])  # [256, 128G] bf16
        rows = meta["h_order"][c * M : (c + 1) * M]
        full[np.ix_(rows, t_cols)] = blk[:, col_src].astype(np.float32)
    return full, res


def kernel(batch_h, batch_t, batch_r, ent_emb, rel_emb):
    score, _ = run(batch_h, batch_t, batch_r, ent_emb, rel_emb)
    return score
